# revision 1
# baseline (speedup 1.0000x reference)
"""Inverse STFT (nn_InverseSTFT) as a Bass/Tile kernel on 8 TRN2 NeuronCores.

Math
----
Reference computes, per batch b:
  full spectrum from one-sided stft via conjugate symmetry (F = 1024),
  ytmp[w, t] = sum_{f,c} full[f, t, c] * basis[f, w, c]          (IDFT)
  y = overlap_add(ytmp, hop=256), window-sum normalize, trim n_fft//2.

Folding the conjugate symmetry into the basis gives an exact K=1024 real
matmul (the imaginary basis rows for f=0 and f=512 are identically zero):
  rows 0..512   : A[f, w]  = cos-basis[f, w] + cos-basis[1024-f, w]   (f=1..511)
  rows 513..1023: Bm[f, w] = im-basis[f, w] - im-basis[1024-f, w]     (f=1..511)
computed with the reference's exact float32 angle arithmetic.

Since hop = 1024/4, write w = 256*j + r. Output sample n = 256*s + r:
  y[256 s + r] = sum_{j=0..3} sum_k basis[k, 256 j + r] * x[k, s - j]
The overlap-add is just PSUM accumulation over 4 frame-shifted matmuls.
Window-sum normalization = multiply by 1/(# valid j), which is 0.25 for
all output segments except s=2 (1/3), s=2000 (1/3), s=2001 (1/2), s=2002 (1).
Output keeps segments s = 2..2002 (trim = first 2 segments).

Sharding: pure data parallel, 2 batches per core.
"""

import numpy as np

import concourse.bass as bass
import concourse.mybir as mybir
from concourse.tile import TileContext
from concourse import bacc, bass_utils

N_FFT = 1024
HOP = 256
B = 16
NFREQ = 513
T = 2000
NCORES = 8
NB = B // NCORES          # batches per core
KC = 8                    # K chunks of 128 (K = 1024)
PAD_L = 3                 # left zero pad (j shifts up to 3)
TPAD = 2056               # 3 + 2000 + 53 (right pad covers last tile reads)
SEG = 2003                # total segments in un-trimmed output
OUT_SEGS = 2001           # segments s = 2..2002
NT = 16                   # s-tiles of 128 per batch (last has 81 valid rows)
OUT_LEN = OUT_SEGS * HOP  # 512256

F32 = mybir.dt.float32

# Matmul input dtype: bfloat16 halves stft/basis DMA traffic and enables
# fast weight load (FWL) on the PE; accumulation stays fp32 in PSUM.
# Validated rel-err vs reference: f32 1.6e-6, bf16 2.1e-3.
import os as _os

USE_BF16 = _os.environ.get("ISTFT_BF16", "1") == "1"
DT_IN = mybir.dt.bfloat16 if USE_BF16 else F32

import ml_dtypes

NP_IN = ml_dtypes.bfloat16 if USE_BF16 else np.float32


def _make_basis() -> np.ndarray:
    """(1024, 1024) folded basis, matching reference's float32 angle math."""
    f = np.arange(N_FFT, dtype=np.float32)
    w = np.arange(N_FFT, dtype=np.float32)
    a32 = np.float32(2.0 * np.pi / N_FFT)
    t1 = (a32 * f).astype(np.float32)
    ang = (t1[:, None] * w[None, :]).astype(np.float32)
    reb = (np.cos(ang).astype(np.float32) / np.float32(N_FFT)).astype(np.float32)
    imb = (-np.sin(ang).astype(np.float32) / np.float32(N_FFT)).astype(np.float32)
    A = np.empty((NFREQ, N_FFT), np.float32)
    A[0] = reb[0]
    A[512] = reb[512]
    A[1:512] = reb[1:512] + reb[1023:512:-1]
    Bm = (imb[1:512] - imb[1023:512:-1]).astype(np.float32)
    return np.concatenate([A, Bm], axis=0)


def _make_scales() -> np.ndarray:
    """(128, 2) per-partition wss fixup (on top of the 0.25 folded into basis).

    col 0 -> first s-tile (s = 2..129): s=2 has 3 frames -> 4/3.
    col 1 -> last s-tile (s = 1922..2002): s=2000 -> 4/3, 2001 -> 2, 2002 -> 4.
    """
    sc = np.ones((128, 2), np.float32)
    sc[0, 0] = np.float32(4.0) / np.float32(3.0)
    sc[78, 1] = np.float32(4.0) / np.float32(3.0)
    sc[79, 1] = 2.0
    sc[80, 1] = 4.0
    return sc


def _prep_x(stft: np.ndarray) -> np.ndarray:
    """(16,513,2000,2) f32 -> (16, KC, 128, TPAD) K-major, t zero-padded."""
    re = stft[:, :, :, 0]                  # (B, 513, T)
    im = stft[:, 1:512, :, 1]              # (B, 511, T)
    xk = np.concatenate([re, im], axis=1)  # (B, 1024, T)
    X = np.zeros((B, N_FFT, TPAD), np.float32)
    X[:, :, PAD_L : PAD_L + T] = xk
    return np.ascontiguousarray(X.reshape(B, KC, 128, TPAD))


def _build_nc() -> bass.Bass:
    nc = bacc.Bacc()
    x_in = nc.dram_tensor("x_in", [NB, KC, 128, TPAD], DT_IN, kind="ExternalInput")
    basis_in = nc.dram_tensor("basis_in", [KC, 128, N_FFT], DT_IN, kind="ExternalInput")
    scale_in = nc.dram_tensor("scale_in", [128, 2], F32, kind="ExternalInput")
    out = nc.dram_tensor("out", [NB, OUT_SEGS, HOP], F32, kind="ExternalOutput")

    with TileContext(nc) as tc:
        with (
            tc.tile_pool(name="xp", bufs=1) as x_pool,
            tc.tile_pool(name="bp", bufs=1) as b_pool,
            tc.tile_pool(name="sp", bufs=1) as s_pool,
            tc.tile_pool(name="ev", bufs=4) as ev_pool,
            tc.tile_pool(name="ps", bufs=4, space="PSUM") as psum_pool,
        ):
            # x chunks issue first on the Sync HWDGE queues (the first
            # matmul's critical path); basis + scale go via GpSimd so the
            # two DMA instruction streams issue in parallel.
            x_sb = [[None] * KC for _ in range(NB)]
            for b in range(NB):
                for kc in range(KC):
                    xt = x_pool.tile(
                        [128, TPAD], DT_IN, name=f"x{b}_{kc}", tag=f"x{b}_{kc}"
                    )
                    nc.sync.dma_start(xt[:, :], x_in[b, kc])
                    x_sb[b][kc] = xt

            basis_sb = []
            for kc in range(KC):
                bt = b_pool.tile([128, N_FFT], DT_IN, name=f"bas{kc}", tag=f"bas{kc}")
                nc.gpsimd.dma_start(bt[:, :], basis_in[kc])
                basis_sb.append(bt)

            scale_sb = s_pool.tile([128, 2], F32, name="scale_sb", tag="scale_sb")
            scale_wu = s_pool.tile([128, 2], F32, name="scale_wu", tag="scale_wu")
            nc.gpsimd.dma_start(scale_sb[:, :], scale_in[:, :])
            # ACT warm-up read of the scale table so later edge-tile
            # activations on ScalarE don't each need the DMA-sem wait.
            nc.scalar.copy(scale_wu[:, :], scale_sb[:, :])

            for b in range(NB):
                for st in range(NT):
                    s0 = 2 + 128 * st
                    psum = psum_pool.tile([128, HOP], F32, name="psum", tag="psum")
                    first = True
                    for kc in range(KC):
                        for j in range(4):
                            c0 = s0 - j + PAD_L
                            nc.tensor.matmul(
                                psum[:, :],
                                x_sb[b][kc][:, c0 : c0 + 128],
                                basis_sb[kc][:, HOP * j : HOP * (j + 1)],
                                start=first,
                                stop=(kc == KC - 1 and j == 3),
                            )
                            first = False
                    # basis is pre-scaled by 0.25 (the steady-state 1/wss);
                    # the two edge tiles apply a per-partition fixup scale
                    # via ScalarE's activation scale vector.
                    ev = ev_pool.tile([128, HOP], F32, name="ev", tag="ev")
                    if st == 0:
                        nc.scalar.mul(ev[:, :], psum[:, :], scale_sb[:, 0:1])
                    elif st == NT - 1:
                        nc.scalar.mul(ev[:, :], psum[:, :], scale_sb[:, 1:2])
                    else:
                        nc.vector.tensor_copy(ev[:, :], psum[:, :])
                    rows = min(128, SEG - s0)
                    nc.sync.dma_start(
                        out[b, 128 * st : 128 * st + rows, :], ev[:rows, :]
                    )
    nc.finalize()
    return nc


def _run(inputs: dict, trace: bool = False):
    stft = np.asarray(inputs["stft_matrix"], dtype=np.float32)
    X = np.ascontiguousarray(_prep_x(stft).astype(NP_IN))
    basis = np.ascontiguousarray(
        (_make_basis() * np.float32(0.25)).reshape(KC, 128, N_FFT).astype(NP_IN)
    )

    scales = _make_scales()
    in_maps = [
        {"x_in": X[NB * c : NB * (c + 1)], "basis_in": basis, "scale_in": scales}
        for c in range(NCORES)
    ]
    nc = _build_nc()
    res = bass_utils.run_bass_kernel_spmd(
        nc, in_maps, core_ids=list(range(NCORES)), trace=trace
    )
    out = np.concatenate(
        [res.results[c]["out"].reshape(NB, OUT_LEN) for c in range(NCORES)], axis=0
    )
    return out, res


def kernel(**inputs) -> np.ndarray:
    out, _ = _run(inputs, trace=False)
    return out



# revision 2
# speedup vs baseline: 1.1581x; 1.1581x over previous
"""Inverse STFT (nn_InverseSTFT) as a Bass/Tile kernel on 8 TRN2 NeuronCores.

Math
----
Reference: full spectrum via conjugate symmetry (F = 1024), IDFT per frame,
overlap-add with hop 256, window-sum normalize, trim n_fft//2.

Since hop = N/4, basis[f, 256j+r] = i^(fj) * basis[f, r] exactly, so
  y[256m + r] = (1/N) Re{ sum_f Z[f,m] e^(2*pi*i*f*r/N) },
  Z[f,m] = sum_{j=0..3} i^(fj) X[f, m-j].
Z preserves conjugate symmetry, so y folds to ONE real K=1024 x 256-wide
matmul per output segment (4x less PE work than matmul-per-shift).
Z is computed on the Vector engine as two shifted-add passes:
  Z2[f,c] = X[f,c] + (-1)^f X[f,c-2]
  Z [f,c] = Z2[f,c] + i^f Z2[f,c-1]
In the folded-real row space, i^f is a per-partition +-1 with Re<->Im row
swaps for odd f; rows are grouped into 8 chunks of 128 so every pass is one
scalar_tensor_tensor (out = (in0 * sgn) + in1) on partition-aligned tiles:
  C0: Re f=2p    C1: Re f=256+2p  C2: [Re512, Im f=2p]  C3: Im f=256+2p
  C4: Re f=2p+1  C5: Im f=2p+1    C6: Re f=257+2p       C7: Im f=257+2p
Window-sum normalization: basis pre-scaled by 0.25; edge segments fixed up
(m=2: 4/3, m=2000: 4/3, m=2001: 2, m=2002: 4) on the output columns.

Layout: output computed as y[r, m] (r in partitions, m streaming) so the
basis is the stationary matmul operand. DRAM tensors use long contiguous
rows (16 KB input strips, 4 KB output rows) — the baseline was DMA
descriptor-rate limited, not bandwidth limited.

Sharding: pure data parallel, 2 batches per core.
"""

import numpy as np
import ml_dtypes

import concourse.bass as bass
import concourse.mybir as mybir
from concourse.tile import TileContext
from concourse import bacc, bass_utils

N_FFT = 1024
HOP = 256
B = 16
NFREQ = 513
T = 2000
NCORES = 8
NB = B // NCORES          # batches per core
TPAD = 2004               # c = t + 1, t in [-1, 2003)
NSTRIP = 2 * NB           # strips per core: (batch, halfK) pairs
STRIPC = 4 * TPAD         # 4 chunks of TPAD cols per strip
OUT_COLS = 2001           # segments m = 2..2002
OUT_PAD = 2048            # padded DRAM row (4 KB bf16)
SC_SIZES = (512, 512, 512, 465)  # psum column chunks over 2001
OUT_LEN = OUT_COLS * HOP  # 512256

F32 = mybir.dt.float32
BF16 = mybir.dt.bfloat16
NP_BF16 = ml_dtypes.bfloat16
MULT = mybir.AluOpType.mult
ADD = mybir.AluOpType.add

# Z-pass partner chunk + sign column within each strip:
# strip g0 = chunks C0..C3 (even f, partner self, sign alt),
# strip g1 = C4..C7 (odd f, partner Re<->Im, signs -alt/+alt).
# sgn columns: 0 -> alt = (-1)^p, 1 -> -alt.
Z_PART = {0: [(0, 0), (1, 0), (2, 0), (3, 0)],
          1: [(1, 1), (0, 0), (3, 1), (2, 0)]}


def _row_map():
    rows = []
    rows += [(2 * p, 0) for p in range(128)]                   # C0
    rows += [(256 + 2 * p, 0) for p in range(128)]             # C1
    rows += [(512, 0)] + [(2 * p, 1) for p in range(1, 128)]   # C2
    rows += [(256 + 2 * p, 1) for p in range(128)]             # C3
    rows += [(2 * p + 1, 0) for p in range(128)]               # C4
    rows += [(2 * p + 1, 1) for p in range(128)]               # C5
    rows += [(257 + 2 * p, 0) for p in range(128)]             # C6
    rows += [(257 + 2 * p, 1) for p in range(128)]             # C7
    return rows


def _make_basis() -> np.ndarray:
    """[128, 2048] bf16: chunk ch at cols 256*ch, Bz[k, r] with conj-sym
    fold (alpha), 1/N, and the 0.25 steady-state wss normalization."""
    r = np.arange(HOP, dtype=np.float32)
    a32 = np.float32(2.0 * np.pi / N_FFT)
    Bz = np.empty((N_FFT, HOP), np.float32)
    for k, (f, c) in enumerate(_row_map()):
        ang = (np.float32(a32 * np.float32(f)) * r).astype(np.float32)
        alpha = np.float32(1.0 if f in (0, 512) else 2.0)
        v = alpha * np.cos(ang) if c == 0 else -alpha * np.sin(ang)
        Bz[k] = v / np.float32(N_FFT) * np.float32(0.25)
    big = Bz.reshape(8, 128, HOP).transpose(1, 0, 2).reshape(128, 8 * HOP)
    return np.ascontiguousarray(big.astype(NP_BF16))


def _make_sgn() -> np.ndarray:
    alt = np.where(np.arange(128) % 2 == 0, 1.0, -1.0).astype(np.float32)
    return np.ascontiguousarray(np.stack([alt, -alt], axis=1))  # [128, 2]


def _prep_x(stft: np.ndarray) -> np.ndarray:
    """(16,513,2000,2) f32 -> (16, 2, 128, STRIPC) bf16 strips, zero padded."""
    rows = _row_map()
    F = np.array([f for f, _ in rows])
    C = np.array([c for _, c in rows])
    xt = stft.transpose(0, 3, 1, 2)          # (B, 2, 513, T)
    data = xt[:, C, F, :]                    # (B, 1024, T)
    X = np.zeros((B, N_FFT, TPAD), NP_BF16)
    X[:, :, 1 : 1 + T] = data.astype(NP_BF16)
    X = X.reshape(B, 2, 4, 128, TPAD).transpose(0, 1, 3, 2, 4)
    return np.ascontiguousarray(X.reshape(B, 2, 128, STRIPC))


def _build_nc() -> bass.Bass:
    nc = bacc.Bacc()
    x_in = nc.dram_tensor("x_in", [NSTRIP, 128, STRIPC], BF16, kind="ExternalInput")
    basis_in = nc.dram_tensor("basis_in", [128, 8 * HOP], BF16, kind="ExternalInput")
    sgn_in = nc.dram_tensor("sgn_in", [128, 2], F32, kind="ExternalInput")
    out = nc.dram_tensor("out", [NB, 2, 128, OUT_PAD], BF16, kind="ExternalOutput")

    with TileContext(nc) as tc:
        with (
            tc.tile_pool(name="xp", bufs=1) as x_pool,
            tc.tile_pool(name="z2p", bufs=1) as z2_pool,
            tc.tile_pool(name="zp", bufs=1) as z_pool,
            tc.tile_pool(name="bp", bufs=1) as b_pool,
            tc.tile_pool(name="evp", bufs=1) as ev_pool,
            tc.tile_pool(name="ps", bufs=1, space="PSUM") as psum_pool,
        ):
            # sgn first on the gpsimd queue (small, gates the Z pass),
            # then basis; x strips stream on the sync queue.
            sgn_sb = b_pool.tile([128, 2], F32, name="sgn_sb", tag="sgn")
            nc.gpsimd.dma_start(sgn_sb[:, :], sgn_in[:, :])
            basis_sb = b_pool.tile([128, 8 * HOP], BF16, name="basis_sb", tag="basis")
            nc.gpsimd.dma_start(basis_sb[:, :], basis_in[:, :])

            x_sb = {}
            for b in range(NB):
                for g in range(2):
                    xt = x_pool.tile([128, STRIPC], BF16, name=f"x{b}_{g}",
                                     tag=f"x{b}_{g}")
                    nc.sync.dma_start(xt[:, :], x_in[2 * b + g])
                    x_sb[b, g] = xt

            z_sb = {}
            for b in range(NB):
                # Z2 then Z, strip g0 first so the first 4 matmul chunks can
                # start while the DVE still works on strip g1.
                z2 = {}
                for g in range(2):
                    z2t = z2_pool.tile([128, STRIPC], BF16, name=f"z2_{g}",
                                       tag=f"z2_{g}")
                    s2 = 1.0 if g == 0 else -1.0
                    xt = x_sb[b, g]
                    for i in range(4):
                        o = TPAD * i
                        nc.vector.scalar_tensor_tensor(
                            out=z2t[:, o + 2 : o + TPAD],
                            in0=xt[:, o : o + TPAD - 2],
                            scalar=s2,
                            in1=xt[:, o + 2 : o + TPAD],
                            op0=MULT, op1=ADD,
                        )
                    z2[g] = z2t
                for g in range(2):
                    zt = z_pool.tile([128, STRIPC], BF16, name=f"z{b}_{g}",
                                     tag=f"z{b}_{g}")
                    for i in range(4):
                        src, scol = Z_PART[g][i]
                        o = TPAD * i
                        os = TPAD * src
                        nc.vector.scalar_tensor_tensor(
                            out=zt[:, o + 3 : o + TPAD],
                            in0=z2[g][:, os + 2 : os + TPAD - 1],
                            scalar=sgn_sb[:, scol : scol + 1],
                            in1=z2[g][:, o + 3 : o + TPAD],
                            op0=MULT, op1=ADD,
                        )
                    z_sb[b, g] = zt

            for b in range(NB):
                ev = {}
                for h in range(2):
                    evt = ev_pool.tile([128, OUT_PAD], BF16, name=f"ev{b}_{h}",
                                       tag=f"ev{b}_{h}")
                    nc.vector.memset(evt[:, OUT_COLS:OUT_PAD], 0.0)
                    ev[h] = evt
                for sc in range(4):
                    cols = SC_SIZES[sc]
                    for h in range(2):
                        ps = psum_pool.tile([128, 512], F32, name="ps",
                                            tag=f"ps{sc}_{h}")
                        for ch in range(8):
                            g, i = divmod(ch, 4)
                            o = TPAD * i + 3 + 512 * sc
                            nc.tensor.matmul(
                                ps[:, :cols],
                                basis_sb[:, HOP * ch + 128 * h : HOP * ch + 128 * h + 128],
                                z_sb[b, g][:, o : o + cols],
                                start=(ch == 0),
                                stop=(ch == 7),
                            )
                        evt = ev[h]
                        nc.scalar.copy(evt[:, 512 * sc : 512 * sc + cols],
                                       ps[:, :cols])
                        if sc == 0:  # m=2 has 3 frames
                            nc.scalar.mul(evt[:, 0:1], ps[:, 0:1], 4.0 / 3.0)
                        elif sc == 3:  # m=2000,2001,2002
                            nc.scalar.mul(evt[:, 1998:1999], ps[:, 462:463], 4.0 / 3.0)
                            nc.scalar.mul(evt[:, 1999:2000], ps[:, 463:464], 2.0)
                            nc.scalar.mul(evt[:, 2000:2001], ps[:, 464:465], 4.0)
                for h in range(2):
                    nc.sync.dma_start(out[b, h], ev[h][:, :])
    nc.finalize()
    return nc


def _run(inputs: dict, trace: bool = False):
    stft = np.asarray(inputs["stft_matrix"], dtype=np.float32)
    X = _prep_x(stft)                        # (16, 2, 128, STRIPC)
    basis = _make_basis()
    sgn = _make_sgn()
    in_maps = [
        {
            "x_in": np.ascontiguousarray(
                X[NB * c : NB * (c + 1)].reshape(NSTRIP, 128, STRIPC)
            ),
            "basis_in": basis,
            "sgn_in": sgn,
        }
        for c in range(NCORES)
    ]
    nc = _build_nc()
    res = bass_utils.run_bass_kernel_spmd(
        nc, in_maps, core_ids=list(range(NCORES)), trace=trace
    )
    outs = []
    for c in range(NCORES):
        o = np.asarray(res.results[c]["out"])  # (NB, 2, 128, OUT_PAD) bf16
        o = o.reshape(NB, 2 * 128, OUT_PAD)[:, :, :OUT_COLS].astype(np.float32)
        outs.append(np.ascontiguousarray(o.transpose(0, 2, 1)).reshape(NB, OUT_LEN))
    return np.concatenate(outs, axis=0), res


def kernel(**inputs) -> np.ndarray:
    out, _ = _run(inputs, trace=False)
    return out


# revision 4
# speedup vs baseline: 1.9693x; 1.7005x over previous
"""Inverse STFT (nn_InverseSTFT) as a Bass/Tile kernel on 8 TRN2 NeuronCores.

Math
----
Reference: full spectrum via conjugate symmetry (F = 1024), IDFT per frame,
overlap-add with hop 256, window-sum normalize, trim n_fft//2.

Since hop = N/4, basis[f, 256j+r] = i^(fj) * basis[f, r] exactly, so
  y[256m + r] = (1/N) Re{ sum_f Z[f,m] e^(2*pi*i*f*r/N) },
  Z[f,m] = sum_{j=0..3} i^(fj) X[f, m-j].
Z preserves conjugate symmetry, so y folds to ONE real K=1024 x 256-wide
matmul per output segment (4x less PE work than matmul-per-shift).
Z is computed on the Vector engine as two shifted-add passes:
  Z2[f,c] = X[f,c] + (-1)^f X[f,c-2]
  Z [f,c] = Z2[f,c] + i^f Z2[f,c-1]
Rows are grouped by f mod 4 into 8 chunks of 128 so each pass is a plain
tensor_tensor add/subtract (uniform op per chunk, partition-aligned
Re<->Im partner chunks for odd f) — the only DVE op family with the
2-byte 2x fast mode; scalar_tensor_tensor has none and is 2x slower.
  C0: Re f=4p   C1: [Re512, Im f=4p]  C2: Re f=4p+2  C3: Im f=4p+2
  C4: Re f=4p+1 C5: Im f=4p+1         C6: Re f=4p+3  C7: Im f=4p+3
Chunks are paired into 4 "units" per batch (C0C1 / C2C3 / C4C5 / C6C7);
units with a uniform op run as one merged DVE instruction (chunk-boundary
columns compute garbage that is never read).
Window-sum normalization: basis pre-scaled by 0.25; edge segments fixed up
(m=2: 4/3, m=2000: 4/3, m=2001: 2, m=2002: 4) on the output columns.

Performance notes (from traces):
- DRAM tensors use long contiguous rows; input split into 8 x 1MB units
  alternating across the two HWDGE queues (sync + scalar) so the first
  unit lands early and DVE starts sooner.
- The PE clock starts in a low p-state; a chained warm-up matmul storm on
  junk data during the DMA lead-in ramps it before real work arrives.
- Output DMA'd per 512-column piece right after its PSUM evacuation to
  keep the drain tail short.

Sharding: pure data parallel, 2 batches per core.
"""

import numpy as np
import ml_dtypes

import concourse.bass as bass
import concourse.mybir as mybir
from concourse.tile import TileContext
from concourse import bacc, bass_utils

N_FFT = 1024
HOP = 256
B = 16
T = 2000
NCORES = 8
NB = B // NCORES          # batches per core
TPAD = 2004               # c = t + 1, t in [-1, 2003)
UNITC = 2 * TPAD          # unit = 2 chunks of TPAD cols
OUT_COLS = 2001           # segments m = 2..2002
OUT_PAD = 2048            # padded DRAM row (4 KB bf16)
SC_SIZES = (512, 512, 512, 465)  # psum column chunks over 2001
OUT_LEN = OUT_COLS * HOP  # 512256
N_WARM = 20               # PE p-state warm-up matmuls

F32 = mybir.dt.float32
BF16 = mybir.dt.bfloat16
NP_BF16 = ml_dtypes.bfloat16
ADD = mybir.AluOpType.add
SUB = mybir.AluOpType.subtract


def _row_map():
    rows = []
    rows += [(4 * p, 0) for p in range(128)]                   # C0
    rows += [(512, 0)] + [(4 * p, 1) for p in range(1, 128)]   # C1
    rows += [(4 * p + 2, 0) for p in range(128)]               # C2
    rows += [(4 * p + 2, 1) for p in range(128)]               # C3
    rows += [(4 * p + 1, 0) for p in range(128)]               # C4
    rows += [(4 * p + 1, 1) for p in range(128)]               # C5
    rows += [(4 * p + 3, 0) for p in range(128)]               # C6
    rows += [(4 * p + 3, 1) for p in range(128)]               # C7
    return rows


def _make_basis() -> np.ndarray:
    """[128, 2048] bf16: chunk ch at cols 256*ch, Bz[k, r] with conj-sym
    fold (alpha), 1/N, and the 0.25 steady-state wss normalization."""
    r = np.arange(HOP, dtype=np.float32)
    a32 = np.float32(2.0 * np.pi / N_FFT)
    Bz = np.empty((N_FFT, HOP), np.float32)
    for k, (f, c) in enumerate(_row_map()):
        ang = (np.float32(a32 * np.float32(f)) * r).astype(np.float32)
        alpha = np.float32(1.0 if f in (0, 512) else 2.0)
        v = alpha * np.cos(ang) if c == 0 else -alpha * np.sin(ang)
        Bz[k] = v / np.float32(N_FFT) * np.float32(0.25)
    big = Bz.reshape(8, 128, HOP).transpose(1, 0, 2).reshape(128, 8 * HOP)
    return np.ascontiguousarray(big.astype(NP_BF16))


def _prep_x(stft: np.ndarray) -> np.ndarray:
    """(16,513,2000,2) f32 -> (16, 4, 128, UNITC) bf16 units, zero padded."""
    rows = _row_map()
    F = np.array([f for f, _ in rows])
    C = np.array([c for _, c in rows])
    xt = stft.transpose(0, 3, 1, 2)          # (B, 2, 513, T)
    data = xt[:, C, F, :]                    # (B, 1024, T)
    X = np.zeros((B, N_FFT, TPAD), NP_BF16)
    X[:, :, 1 : 1 + T] = data.astype(NP_BF16)
    X = X.reshape(B, 4, 2, 128, TPAD).transpose(0, 1, 3, 2, 4)
    return np.ascontiguousarray(X.reshape(B, 4, 128, UNITC))


def _build_nc() -> bass.Bass:
    nc = bacc.Bacc()
    x_in = nc.dram_tensor("x_in", [4 * NB, 128, UNITC], BF16, kind="ExternalInput")
    basis_in = nc.dram_tensor("basis_in", [128, 8 * HOP], BF16, kind="ExternalInput")
    out = nc.dram_tensor("out", [NB, 2, 128, OUT_PAD], BF16, kind="ExternalOutput")

    with TileContext(nc) as tc:
        with (
            tc.tile_pool(name="xp", bufs=1) as x_pool,
            tc.tile_pool(name="z2p", bufs=1) as z2_pool,
            tc.tile_pool(name="zp", bufs=1) as z_pool,
            tc.tile_pool(name="bp", bufs=1) as b_pool,
            tc.tile_pool(name="evp", bufs=1) as ev_pool,
            tc.tile_pool(name="ps", bufs=1, space="PSUM") as psum_pool,
        ):
            # basis + warm-up junk via gpsimd (SWDGE/Pool, otherwise idle);
            # x units alternate across the two HWDGE queues (sync, scalar).
            basis_sb = b_pool.tile([128, 8 * HOP], BF16, name="basis_sb", tag="basis")
            nc.gpsimd.dma_start(basis_sb[:, :], basis_in[:, :])
            junk = b_pool.tile([128, 512], BF16, name="junk", tag="junk")
            nc.gpsimd.memset(junk[:, :], 0.0)

            # PE p-state warm-up: chained junk matmuls, done before real work.
            ps_w = psum_pool.tile([128, 512], F32, name="psw", tag="ps3_1")
            for w in range(N_WARM):
                nc.tensor.matmul(ps_w[:, :], junk[:, :128], junk[:, :],
                                 start=(w == 0), stop=(w == N_WARM - 1))

            x_sb = {}
            for b in range(NB):
                for u in range(4):
                    xt = x_pool.tile([128, UNITC], BF16, name=f"x{b}_{u}",
                                     tag=f"x{b}_{u}")
                    eng = nc.sync if (2 * b + u) % 2 == 0 else nc.scalar
                    eng.dma_start(xt[:, :], x_in[4 * b + u])
                    x_sb[b, u] = xt

            z_sb = {}
            for b in range(NB):
                z2 = {}
                for u in range(4):
                    z2t = z2_pool.tile([128, UNITC], BF16, name=f"z2_{u}",
                                       tag=f"z2_{u}")
                    xt = x_sb[b, u]
                    # merged Z2 over both chunks; boundary cols unread
                    nc.vector.tensor_tensor(
                        out=z2t[:, 2:UNITC],
                        in0=xt[:, 2:UNITC],
                        in1=xt[:, 0 : UNITC - 2],
                        op=ADD if u < 2 else SUB,
                    )
                    z2[u] = z2t
                for u in range(4):
                    zt = z_pool.tile([128, UNITC], BF16, name=f"z{b}_{u}",
                                     tag=f"z{b}_{u}")
                    if u < 2:
                        # C0C1: Z = Z2[c] + Z2[c-1]; C2C3: minus. Merged:
                        # shift-by-1 stays chunk-aligned across the unit.
                        nc.vector.tensor_tensor(
                            out=zt[:, 3:UNITC],
                            in0=z2[u][:, 3:UNITC],
                            in1=z2[u][:, 2 : UNITC - 1],
                            op=ADD if u == 0 else SUB,
                        )
                    else:
                        # odd f: partner chunk Re<->Im within the unit
                        reo, imo = 0, TPAD
                        op_re, op_im = (SUB, ADD) if u == 2 else (ADD, SUB)
                        nc.vector.tensor_tensor(
                            out=zt[:, reo + 3 : reo + TPAD],
                            in0=z2[u][:, reo + 3 : reo + TPAD],
                            in1=z2[u][:, imo + 2 : imo + TPAD - 1],
                            op=op_re,
                        )
                        nc.vector.tensor_tensor(
                            out=zt[:, imo + 3 : imo + TPAD],
                            in0=z2[u][:, imo + 3 : imo + TPAD],
                            in1=z2[u][:, reo + 2 : reo + TPAD - 1],
                            op=op_im,
                        )
                    z_sb[b, u] = zt

            for b in range(NB):
                ev = {}
                for h in range(2):
                    ev[h] = ev_pool.tile([128, OUT_PAD], BF16, name=f"ev{b}_{h}",
                                         tag=f"ev{b}_{h}")
                for sc in range(4):
                    cols = SC_SIZES[sc]
                    for h in range(2):
                        ps = psum_pool.tile([128, 512], F32, name="ps",
                                            tag=f"ps{sc}_{h}")
                        for ch in range(8):
                            u, i = divmod(ch, 2)
                            o = TPAD * i + 3 + 512 * sc
                            nc.tensor.matmul(
                                ps[:, :cols],
                                basis_sb[:, HOP * ch + 128 * h : HOP * ch + 128 * h + 128],
                                z_sb[b, u][:, o : o + cols],
                                start=(ch == 0),
                                stop=(ch == 7),
                            )
                        evt = ev[h]
                        nc.scalar.copy(evt[:, 512 * sc : 512 * sc + cols],
                                       ps[:, :cols])
                        if sc == 0:  # m=2 has 3 frames
                            nc.scalar.mul(evt[:, 0:1], ps[:, 0:1], 4.0 / 3.0)
                        elif sc == 3:  # m=2000,2001,2002
                            nc.scalar.mul(evt[:, 1998:1999], ps[:, 462:463], 4.0 / 3.0)
                            nc.scalar.mul(evt[:, 1999:2000], ps[:, 463:464], 2.0)
                            nc.scalar.mul(evt[:, 2000:2001], ps[:, 464:465], 4.0)
                        eng = nc.sync if (sc + h) % 2 == 0 else nc.scalar
                        eng.dma_start(
                            out[b, h, :, 512 * sc : 512 * sc + cols],
                            evt[:, 512 * sc : 512 * sc + cols],
                        )
    nc.finalize()
    return nc


def _run(inputs: dict, trace: bool = False):
    stft = np.asarray(inputs["stft_matrix"], dtype=np.float32)
    X = _prep_x(stft)                        # (16, 4, 128, UNITC)
    basis = _make_basis()
    in_maps = [
        {
            "x_in": np.ascontiguousarray(
                X[NB * c : NB * (c + 1)].reshape(4 * NB, 128, UNITC)
            ),
            "basis_in": basis,
        }
        for c in range(NCORES)
    ]
    nc = _build_nc()
    res = bass_utils.run_bass_kernel_spmd(
        nc, in_maps, core_ids=list(range(NCORES)), trace=trace
    )
    outs = []
    for c in range(NCORES):
        o = np.asarray(res.results[c]["out"])  # (NB, 2, 128, OUT_PAD) bf16
        o = o.reshape(NB, 2 * 128, OUT_PAD)[:, :, :OUT_COLS].astype(np.float32)
        outs.append(np.ascontiguousarray(o.transpose(0, 2, 1)).reshape(NB, OUT_LEN))
    return np.concatenate(outs, axis=0), res


def kernel(**inputs) -> np.ndarray:
    out, _ = _run(inputs, trace=False)
    return out
